# revision 28
# baseline (speedup 1.0000x reference)
"""Trainium2 Bass kernel for nn_AOSA_76733885710837 (dense_transformer).

Per-batch attention layer with double-normalized softmax + BatchNorm tail,
data-parallel over batch B=8 across 8 NeuronCores (one batch per core);
the small CxC weights are replicated.

Math restructuring (validated numerically against the reference):
  q = Wq@x, k = Wk@x                      [C, N]
  u = Wt@x                                [C, N]
  wT = x^T @ (Wt@Wv)^T                    [N, C]   (M = Wt@Wv on host)
  E = exp(q^T k - K_SOFT)                 constant shift instead of row max
  rs[n] = sum_m E[n, m]; recip = 1/rs
  wTs[n, o] = wT[n, o] * recip[n]         (folds the row softmax divide)
  colsum[m] = sum_n recip[n] E[n, m]      (bf16 accumulation on DVE)
  rb[m] = 1 / (1e-9 + colsum[m])
  Wt@x_r = (wTs^T @ E) * rb + bw          (bw = Wt@bv; colsum*rb ~= 1)
  z = (u - bw) - (wTs^T E) * rb           so  x_z = alpha*z + (alpha*bt+beta)
  stats: per-channel sum(z), sum(z^2) transformed to sum(xz), sum(xz^2)
         algebraically, AllReduce'd over the 8 cores PER CHANNEL HALF so
         the first AllReduce + half the BN drain hide under the second
         half's attention-apply compute.
  out = x + relu(Af*z + Bf)  with Af/Bf folding alpha/ab/gamma/mean/var.

Key scheduling facts (from perfetto/NTFF traces of earlier versions):
  - matmul cadence is ~263ns per 512-row instruction for f32r AND bf16
    (1 col/cycle both); only fp8+DoubleRow is faster, and fp8 E flushes
    small attention weights to zero which breaks the double
    normalization (measured 10% error). So the PE work is fixed and the
    wins are structural.
  - the Wt@(x-x_r) stage is folded into the apply matmul via M=Wt@Wv,
    so after the last matmul only a short DVE chain + stats remain.
  - one 2KB AllReduce for the BN stats (a split pair serializes on the
    single CC stream and loses); the alpha/beta/bt affine is folded
    into the stats transform and the BN coefficients, so no separate
    x_z pass exists at all.
  - ALL input DMAs ride one queue in consumption order (a single
    dma_start uses all 16 DMA engines; parallel queues split bandwidth
    and starve the projection pipeline).
  - dummy "preheat" matmuls during the DMA window hold the PE p-state
    ramp (cold cadence is ~427-512ns vs 263ns warm).
  - DVE [P,2048] bf16: tensor_scalar/tensor_tensor are the fast paths
    (~0.5-0.7us); scalar_tensor_tensor and reduce are 1 elem/cycle
    (~2.3us) - the colsum accumulation must use the TS+TT pair.
  - exp keeps accum_out for the row sums: every alternative rowsum
    placement (DVE reduce, gpsimd) is slower or unsupported.

NOTE: walrus --enable-ldw-opt=true crashes codegen on the f32r weight
loads (visitInstLdweights) - it must stay off.
"""

import sys

for _p in ("/opt/trn_rl_repo",):
    if _p not in sys.path:
        sys.path.append(_p)

import numpy as np

import concourse.bass as bass
import concourse.mybir as mybir
import concourse.tile as tile
from concourse import bacc
import concourse.bass_utils as _bu
from concourse.bass_utils import run_bass_kernel_spmd

F32 = mybir.dt.float32
F32R = mybir.dt.float32r
BF16 = mybir.dt.bfloat16
AL = mybir.AluOpType
AF = mybir.ActivationFunctionType
AX = mybir.AxisListType

B, C, N = 8, 256, 2048
P = 128
CB = C // P          # 2 channel blocks
NB = N // P          # 16 row blocks
NQ = N // 512        # 4 column chunks of 512
K_SOFT = 64.0
BN_EPS = 1e-5
DENOM = 1.0 / (B * N)
N_CORES = 8

# vpack vector slot indices
VI = {"bt": 0, "gam": 1, "bnb": 2, "al": 3, "be": 4, "bw": 5}
# wpack projection slot indices
WI = {"q": 0, "k": 1, "m": 2, "t": 3}


def _build_body(tc, x_d, w_d, v_d, out_d, dbg=None):
    nc = tc.nc

    def dump(name, ap):
        if dbg is not None and name in dbg:
            nc.sync.dma_start(dbg[name], ap)

    with (
        tc.tile_pool(name="pp", bufs=1) as pp,
        tc.tile_pool(name="wp", bufs=2) as wp,
        tc.tile_pool(name="dramp", bufs=1, space="DRAM") as dramp,
    ):
        # ---- warmup collective ------------------------------------------
        # a tiny AllReduce issued at kernel start absorbs the
        # first-collective CC-stream setup cost concurrently with compute
        warm_s = pp.tile([1, 8], F32)
        nc.vector.memset(warm_s, 0.0)
        warm_in = dramp.tile([1, 8], F32, name="warm_in")
        warm_out = dramp.tile([1, 8], F32, addr_space="Shared", name="warm_out")
        nc.sync.dma_start(warm_in, warm_s)
        nc.gpsimd.collective_compute(
            "AllReduce",
            AL.add,
            replica_groups=[list(range(N_CORES))],
            ins=[warm_in.opt()],
            outs=[warm_out.opt()],
        )

        # ---- input DMAs (chunk-major, >=4KB contiguous runs) ------------
        x_s = pp.tile([P, NQ, CB * 512], F32R, name="x_s")
        xf_s = x_s.bitcast(F32)
        xp = x_d.rearrange("p (q r) -> p q r", q=NQ)
        wpack = pp.tile([P, 4, CB, C], F32R, name="wpack")
        wsrc = w_d.rearrange("p (w cb o) -> p w cb o", w=4, cb=CB)
        vpack = pp.tile([P, 6, CB], F32, name="vpack")

        # ALL input DMAs ride ONE queue in exact consumption order: a single
        # dma_start already spreads its descriptors across all 16 DMA
        # engines at full HBM bandwidth, while parallel queues SPLIT the
        # bandwidth three ways and make the piece the PE needs next arrive
        # 3x later (measured: q/k stalled ~7us waiting for x chunks).
        nc.sync.dma_start(wpack[:, 0:2], wsrc[:, 0:2])            # Wq, Wk
        nc.sync.dma_start(x_s[:, 0], xp[:, 0])
        nc.sync.dma_start(x_s[:, 1], xp[:, 1])
        nc.sync.dma_start(x_s[:, 2], xp[:, 2])
        nc.sync.dma_start(x_s[:, 3], xp[:, 3])
        nc.sync.dma_start(wpack[:, 2:4], wsrc[:, 2:4])            # M, Wt
        nc.sync.dma_start(vpack, v_d.rearrange("p (v cb) -> p v cb", v=6))

        bt_s = vpack[:, VI["bt"]]
        gam_s = vpack[:, VI["gam"]]
        bnb_s = vpack[:, VI["bnb"]]
        al_s = vpack[:, VI["al"]]
        be_s = vpack[:, VI["be"]]
        bw_s = vpack[:, VI["bw"]]

        # ---- constants --------------------------------------------------
        ones_col_b = pp.tile([P, 1], BF16)
        nc.vector.memset(ones_col_b, 1.0)
        negk_bias = pp.tile([P, 1], F32)
        nc.vector.memset(negk_bias, -K_SOFT)
        eps_bias = pp.tile([P, 1], F32)
        nc.vector.memset(eps_bias, BN_EPS)

        # per-channel constants for the stats transform:
        #   s1 = al*sum_z + N*ab
        #   s2 = al^2*sum_z2 + 2*al*ab*sum_z + N*ab^2
        ab_s = pp.tile([P, CB], F32)        # alpha*bt + beta
        al2_s = pp.tile([P, CB], F32)       # alpha^2
        talab_s = pp.tile([P, CB], F32)     # 2*alpha*ab
        c1_s = pp.tile([P, CB], F32)        # N*ab
        c2_s = pp.tile([P, CB], F32)        # N*ab^2
        nc.vector.tensor_tensor(ab_s, al_s, bt_s, AL.mult)
        nc.vector.tensor_tensor(ab_s, ab_s, be_s, AL.add)
        nc.vector.tensor_tensor(al2_s, al_s, al_s, AL.mult)
        nc.vector.tensor_tensor(talab_s, al_s, ab_s, AL.mult)
        nc.vector.tensor_scalar_mul(talab_s, talab_s, 2.0)
        nc.vector.tensor_scalar_mul(c1_s, ab_s, float(N))
        nc.vector.tensor_tensor(c2_s, ab_s, ab_s, AL.mult)
        nc.vector.tensor_scalar_mul(c2_s, c2_s, float(N))

        # ---- P1: projections q, k, u (Wt@x), wT (x^T M^T) ---------------
        # q/k live only through the energy loop; a released pool frees their
        # 32KB/partition for the drain pipeline buffers afterwards
        qkp = tc.alloc_tile_pool(name="qkp", bufs=1)
        q_s = qkp.tile([P, CB, N], F32R, name="q_s")
        k_s = qkp.tile([P, CB, N], F32R, name="k_s")
        u_s = pp.tile([P, CB, N], F32, name="u_s")
        wT_s = pp.tile([P, NB, C], BF16, name="wT_s")
        # PE preheat: dummy matmuls on zeroed scratch keep the Tensor engine
        # busy through the input-DMA window so the p-state ramp (full clock
        # needs ~3us of continuous execution) completes before the real
        # projections start; without it the first ~7 matmuls after every
        # cold start run at the mid p-state (427ns vs 263ns per 512 rows).
        ph_w = pp.tile([P, P], BF16, name="ph_w")
        ph_r = pp.tile([P, 512], BF16, name="ph_r")
        nc.vector.memset(ph_w, 0.0)
        nc.vector.memset(ph_r, 0.0)
        with tc.tile_pool(name="psA", bufs=4, space="PSUM") as psA:
            ph_p = psA.tile([P, 512], F32, tag="ph", bufs=1, name="ph_p")
            for _ in range(8):
                nc.tensor.matmul(ph_p, ph_w, ph_r, start=True, stop=True)
            for ch in range(NQ):
                sl = slice(ch * 512, (ch + 1) * 512)
                for ob in range(CB):
                    pq = psA.tile([P, 512], F32, tag="pqk", name="pq")
                    for ci in range(CB):
                        nc.tensor.matmul(
                            pq,
                            wpack[:, WI["q"], ci, ob * P : (ob + 1) * P],
                            x_s[:, ch, ci * 512 : (ci + 1) * 512],
                            start=(ci == 0),
                            stop=(ci == CB - 1),
                        )
                    pk = psA.tile([P, 512], F32, tag="pqk", name="pk")
                    for ci in range(CB):
                        nc.tensor.matmul(
                            pk,
                            wpack[:, WI["k"], ci, ob * P : (ob + 1) * P],
                            x_s[:, ch, ci * 512 : (ci + 1) * 512],
                            start=(ci == 0),
                            stop=(ci == CB - 1),
                        )
                    nc.any.tensor_copy(q_s[:, ob, sl], pq)
                    nc.any.tensor_copy(k_s[:, ob, sl], pk)
            for ch in range(NQ):
                for j in range(4):
                    nb = ch * 4 + j
                    pw = psA.tile([P, C], F32, tag="pw", bufs=2, name="pw")
                    for ci in range(CB):
                        nc.tensor.matmul(
                            pw,
                            x_s[:, ch, ci * 512 + j * P : ci * 512 + (j + 1) * P],
                            wpack[:, WI["m"], ci, :],
                            start=(ci == 0),
                            stop=(ci == CB - 1),
                        )
                    nc.any.tensor_copy(wT_s[:, nb, :], pw)

        dump("q_s", q_s)
        dump("k_s", k_s)
        dump("u_s", u_s)
        dump("wT_s", wT_s)

        # ---- P2: energy -> exp -> row normalizers -----------------------
        E_s = pp.tile([P, NB, N], BF16, name="E_s")
        wTs_s = pp.tile([P, NB, C], BF16, name="wTs_s")
        acc_s = pp.tile([P, N], BF16, name="acc_s")
        recip_s = pp.tile([P, NB], F32, name="recip_s")
        with tc.tile_pool(name="psE", bufs=2, space="PSUM") as psE:
            for i in range(NB):
                pe = psE.tile([P, N], F32, tag="e", name="pe")
                for cb in range(CB):
                    for qd in range(NQ):
                        nc.tensor.matmul(
                            pe[:, qd * 512 : (qd + 1) * 512],
                            q_s[:, cb, i * P : (i + 1) * P],
                            k_s[:, cb, qd * 512 : (qd + 1) * 512],
                            start=(cb == 0),
                            stop=(cb == CB - 1),
                        )
                # rowsum via the activation accumulator (a DVE reduce of
                # [P,2048]bf16 is the SLOW 1-elem/cycle path - 2.3us - and
                # GpSimd only reduces along partitions); the TS+TT pair for
                # the colsum accumulation is likewise 2x faster than the
                # fused scalar_tensor_tensor on this input shape.
                rs = wp.tile([P, 1], F32, tag="rs", name="rs")
                nc.scalar.activation(
                    E_s[:, i, :], pe, AF.Exp, bias=negk_bias, accum_out=rs
                )
                nc.vector.reciprocal_approx_fast(recip_s[:, i : i + 1], rs)
                nc.vector.tensor_scalar_mul(
                    wTs_s[:, i, :], wT_s[:, i, :], recip_s[:, i : i + 1]
                )
                if i == 0:
                    nc.vector.tensor_scalar(
                        acc_s, E_s[:, i, :], recip_s[:, i : i + 1], None, AL.mult
                    )
                else:
                    En = wp.tile([P, N], BF16, tag="En", name="En")
                    nc.vector.tensor_scalar(
                        En, E_s[:, i, :], recip_s[:, i : i + 1], None, AL.mult
                    )
                    nc.vector.tensor_tensor(acc_s, acc_s, En, AL.add)

            # u projection INSIDE the psE pool, on tag-"e" tiles: the pool
            # cycles 2 buffers, so these allocations wait only on exp_14's
            # buffer (free ~2us before exp_15's) and the PE rolls straight
            # through the energy->apply boundary instead of idling ~2.7us
            # on the exp_15 PSUM handoff.
            for ch in range(NQ):
                usl = slice(ch * 512, (ch + 1) * 512)
                for ob in range(CB):
                    pu = psE.tile([P, N], F32, tag="e", name="pu")
                    for ci in range(CB):
                        nc.tensor.matmul(
                            pu[:, 0:512],
                            wpack[:, WI["t"], ci, ob * P : (ob + 1) * P],
                            x_s[:, ch, ci * 512 : (ci + 1) * 512],
                            start=(ci == 0),
                            stop=(ci == CB - 1),
                        )
                    nc.any.tensor_copy(u_s[:, ob, usl], pu[:, 0:512])

        dump("E_s", E_s)
        dump("wTs_s", wTs_s)
        dump("recip_s", recip_s)

        qkp.release()

        with tc.tile_pool(name="psX", bufs=4, space="PSUM") as psX:
            rb_s = pp.tile([P, N], F32, name="rb_s")
            z_s = pp.tile([P, CB, N], F32, name="z_s")
            zl1 = pp.tile([P, CB, NQ], F32, name="zl1")
            zl2 = pp.tile([P, CB, NQ], F32, name="zl2")

            def apply_mm(ob, qd):
                sl = slice(qd * 512, (qd + 1) * 512)
                px = psX.tile([P, 512], F32, tag="px", name="px")
                for i in range(NB):
                    nc.tensor.matmul(
                        px,
                        wTs_s[:, i, ob * P : (ob + 1) * P],
                        E_s[:, i, sl],
                        start=(i == 0),
                        stop=(i == NB - 1),
                    )
                return px

            def apply_ep(ob, qd, px):
                sl = slice(qd * 512, (qd + 1) * 512)
                t1 = wp.tile([P, 512], F32, tag="t1", bufs=3, name="t1")
                nc.vector.tensor_tensor(t1, px, rb_s[:, sl], AL.mult)
                # z = (u - bw) - t1 ; accumulate per-channel sum(z)
                nc.vector.scalar_tensor_tensor(
                    z_s[:, ob, sl],
                    u_s[:, ob, sl],
                    bw_s[:, ob : ob + 1],
                    t1,
                    AL.subtract,
                    AL.subtract,
                    accum_out=zl1[:, ob, qd : qd + 1],
                )
                # sum(z^2) on the Scalar engine (idle during apply)
                sq = wp.tile([P, 512], F32, tag="sq", bufs=3, name="sq")
                nc.scalar.activation(
                    sq,
                    z_s[:, ob, sl],
                    AF.Square,
                    accum_out=zl2[:, ob, qd : qd + 1],
                )

            # the first two apply chunks' matmuls are emitted before the
            # column-normalizer work so the PE rolls straight into the apply
            # chain and the colsum matmuls (which wait on the DVE acc chain)
            # never bubble the PE (the epilogues, which read rb_s, are
            # emitted after colsum so Tile orders the writes first)
            px00 = apply_mm(0, 0)
            px01 = apply_mm(0, 1)

            # ---- column normalizer rb = 1/(1e-9 + colsum), broadcast ----
            for qd in range(NQ):
                sl = slice(qd * 512, (qd + 1) * 512)
                pcs = psX.tile([1, 512], F32, tag="cs", bufs=2, name="pcs")
                nc.tensor.matmul(pcs, ones_col_b, acc_s[:, sl], start=True, stop=True)
                rt = wp.tile([1, 512], F32, tag="rt", bufs=1, name="rt")
                nc.vector.tensor_scalar_add(rt, pcs, 1e-9)
                rb1 = wp.tile([1, 512], F32, tag="rb1", bufs=1, name="rb1")
                nc.vector.reciprocal_approx_fast(rb1, rt)
                nc.gpsimd.partition_broadcast(rb_s[:, sl], rb1)

            dump("rb_s", rb_s)

            # ---- apply + per-channel-half stats, pipelined AllReduces ---
            # AR(ob0) triggers ~17us before AR(ob1): it absorbs the core-
            # launch skew + ring latency while the ob1 apply matmuls run, so
            # AR(ob1) starts on a free CC stream between already-synced
            # cores (~14us ring only). The ob0 BN drain then hides inside
            # AR(ob1)'s window; only the ob1 half-drain stays exposed.
            # Partial stat reduces for qd0..2 are emitted before the last
            # chunk so the post-last-matmul chain is just add+3xSTT+DMA.
            stats_t, sin_d, sout_d = [], [], []
            for ob in range(CB):
                stats_t.append(pp.tile([P, 2], F32, name=f"stats{ob}"))
                sin_d.append(dramp.tile([P, 2], F32, name=f"sin{ob}"))
                sout_d.append(
                    dramp.tile([P, 2], F32, addr_space="Shared", name=f"sout{ob}")
                )
            szp = pp.tile([P, 2 * CB], F32, name="szp")  # partial/final sums

            def ob_stats(ob):
                # finalize sums: partial(qd0..2) + last chunk's column
                nc.vector.tensor_tensor(
                    szp[:, 2 * ob : 2 * ob + 1],
                    szp[:, 2 * ob : 2 * ob + 1],
                    zl1[:, ob, 3:4],
                    AL.add,
                )
                nc.vector.tensor_tensor(
                    szp[:, 2 * ob + 1 : 2 * ob + 2],
                    szp[:, 2 * ob + 1 : 2 * ob + 2],
                    zl2[:, ob, 3:4],
                    AL.add,
                )
                st = stats_t[ob]
                szt = szp[:, 2 * ob : 2 * ob + 1]
                sz2t = szp[:, 2 * ob + 1 : 2 * ob + 2]
                # s1 = al*sum_z + N*ab
                nc.vector.scalar_tensor_tensor(
                    st[:, 0:1], szt, al_s[:, ob : ob + 1], c1_s[:, ob : ob + 1],
                    AL.mult, AL.add,
                )
                # s2 = al^2*sum_z2 + (2*al*ab*sum_z + N*ab^2)
                ta = wp.tile([P, 1], F32, tag="ta", name="ta")
                nc.vector.scalar_tensor_tensor(
                    ta, szt, talab_s[:, ob : ob + 1], c2_s[:, ob : ob + 1],
                    AL.mult, AL.add,
                )
                nc.vector.scalar_tensor_tensor(
                    st[:, 1:2], sz2t, al2_s[:, ob : ob + 1], ta, AL.mult, AL.add
                )
                nc.gpsimd.dma_start(sin_d[ob], st)
                nc.gpsimd.collective_compute(
                    "AllReduce",
                    AL.add,
                    replica_groups=[list(range(N_CORES))],
                    ins=[sin_d[ob].opt()],
                    outs=[sout_d[ob].opt()],
                )

            apply_ep(0, 0, px00)
            apply_ep(0, 1, px01)
            for ob in range(CB):
                for qd in range(NQ):
                    if ob == 0 and qd < 2:
                        continue
                    apply_ep(ob, qd, apply_mm(ob, qd))
                    if qd == 2:
                        # partial sums over qd0..2 while qd3's matmuls run
                        nc.vector.reduce_sum(
                            szp[:, 2 * ob : 2 * ob + 1], zl1[:, ob, 0:3], axis=AX.X
                        )
                        nc.vector.reduce_sum(
                            szp[:, 2 * ob + 1 : 2 * ob + 2],
                            zl2[:, ob, 0:3],
                            axis=AX.X,
                        )
                ob_stats(ob)

            dump("z_s", z_s)

            # ---- BN coefficients + drain, per channel half --------------
            # emitted AFTER both apply halves so the drain-ob0 instructions
            # never head-of-line-block the ob1 epilogues; at execution time
            # drain-ob0 runs during AR(ob1)'s window.
            op = out_d.rearrange("p (cb n) -> p cb n", cb=CB)
            Af_s = pp.tile([P, CB], F32, name="Af_s")
            Bf_s = pp.tile([P, CB], F32, name="Bf_s")
            with tc.tile_pool(name="ep", bufs=8) as ep:
                for ob in range(CB):
                    rstat = pp.tile([P, 2], F32, name=f"rstat{ob}")
                    nc.sync.dma_start(rstat, sout_d[ob])
                    mv = wp.tile([P, 2], F32, tag="mv", name="mv")
                    nc.vector.tensor_scalar_mul(mv, rstat, DENOM)
                    mean = mv[:, 0:1]
                    m2 = wp.tile([P, 1], F32, tag="m2", name="m2")
                    nc.vector.tensor_tensor(m2, mean, mean, AL.mult)
                    varr = wp.tile([P, 1], F32, tag="varr", name="varr")
                    nc.vector.tensor_tensor(varr, mv[:, 1:2], m2, AL.subtract)
                    sd = wp.tile([P, 1], F32, tag="sd", name="sd")
                    nc.scalar.activation(sd, varr, AF.Sqrt, bias=eps_bias)
                    inv = wp.tile([P, 1], F32, tag="inv", name="inv")
                    nc.vector.reciprocal(inv, sd)
                    a0 = wp.tile([P, 1], F32, tag="a0", name="a0")
                    nc.vector.tensor_tensor(a0, gam_s[:, ob : ob + 1], inv, AL.mult)
                    nc.vector.tensor_tensor(
                        Af_s[:, ob : ob + 1], a0, al_s[:, ob : ob + 1], AL.mult
                    )
                    tmb = wp.tile([P, 1], F32, tag="tmb", name="tmb")
                    nc.vector.tensor_tensor(
                        tmb, ab_s[:, ob : ob + 1], mean, AL.subtract
                    )
                    nc.vector.tensor_tensor(tmb, a0, tmb, AL.mult)
                    nc.vector.tensor_tensor(
                        Bf_s[:, ob : ob + 1], tmb, bnb_s[:, ob : ob + 1], AL.add
                    )
                    for qd in range(NQ):
                        sl = slice(qd * 512, (qd + 1) * 512)
                        xn = ep.tile([P, 512], F32, tag="xn", name="xn")
                        nc.scalar.activation(
                            xn,
                            z_s[:, ob, sl],
                            AF.Relu,
                            bias=Bf_s[:, ob : ob + 1],
                            scale=Af_s[:, ob : ob + 1],
                        )
                        oc = ep.tile([P, 512], F32, tag="oc", name="oc")
                        nc.vector.tensor_tensor(
                            oc, xn, xf_s[:, qd, ob * 512 : (ob + 1) * 512], AL.add
                        )
                        (nc.sync if qd % 2 == 0 else nc.gpsimd).dma_start(
                            op[:, ob, sl], oc
                        )


def build():
    nc = bacc.Bacc(
        "TRN2", target_bir_lowering=False, debug=False, num_devices=N_CORES
    )
    x_d = nc.dram_tensor("x", [P, NQ * CB * 512], F32R, kind="ExternalInput").ap()
    w_d = nc.dram_tensor("wpack", [P, 4 * CB * C], F32R, kind="ExternalInput").ap()
    v_d = nc.dram_tensor("vpack", [P, 6 * CB], F32, kind="ExternalInput").ap()
    out_d = nc.dram_tensor("out", [P, CB * N], F32, kind="ExternalOutput").ap()
    with tile.TileContext(nc) as tc:
        _build_body(tc, x_d, w_d, v_d, out_d)
    nc.compile()
    return nc


_NC_CACHE = None


def _get_nc():
    global _NC_CACHE
    if _NC_CACHE is None:
        _NC_CACHE = build()
    return _NC_CACHE


def pack_inputs(inputs):
    f = lambda k: np.asarray(inputs[k], dtype=np.float32)
    x = f("x")
    # [C, N] -> [P, NQ, CB, 512] chunk-major -> [P, NQ*CB*512]
    xp = [
        np.ascontiguousarray(
            x[b]
            .reshape(CB, P, NQ, 512)
            .transpose(1, 2, 0, 3)
            .reshape(P, NQ * CB * 512)
        )
        for b in range(B)
    ]
    Wt64 = f("Wt").astype(np.float64)
    M = (Wt64 @ f("Wv").astype(np.float64)).astype(np.float32)
    bw = (Wt64 @ f("bv").astype(np.float64)).astype(np.float32)
    wts = np.stack([f("Wq").T, f("Wk").T, M.T, f("Wt").T])  # [4, C(in), C(out)]
    wpack = np.ascontiguousarray(
        wts.reshape(4, CB, P, C).transpose(2, 0, 1, 3).reshape(P, 4 * CB * C)
    )
    vecs = np.stack(
        [
            f("bt"),
            f("bn_gamma"),
            f("bn_beta"),
            f("alpha").reshape(C),
            f("beta").reshape(C),
            bw,
        ]
    )  # [6, C]
    vpack = np.ascontiguousarray(
        vecs.reshape(6, CB, P).transpose(2, 0, 1).reshape(P, 6 * CB)
    )
    shared = {"wpack": wpack, "vpack": vpack}
    return xp, shared


def kernel(**inputs):
    xp, shared = pack_inputs(inputs)
    nc = _get_nc()
    in_maps = [dict(shared, x=xp[b]) for b in range(B)]
    res = run_bass_kernel_spmd(nc, in_maps, core_ids=list(range(N_CORES)))
    out = np.stack([res.results[b]["out"] for b in range(B)], axis=0)
    # [B, P, CB*N] -> [B, C, N]
    return np.ascontiguousarray(
        out.reshape(B, P, CB, N).transpose(0, 2, 1, 3).reshape(B, C, N)
    )


# revision 29
# speedup vs baseline: 1.4494x; 1.4494x over previous
"""Trainium2 Bass kernel for nn_AOSA_76733885710837 (dense_transformer).

Per-batch attention layer with double-normalized softmax + BatchNorm tail,
data-parallel over batch B=8 across 8 NeuronCores (one batch per core);
the small CxC weights are replicated.

Math restructuring (validated numerically against the reference):
  q = Wq@x, k = Wk@x                      [C, N]
  u = Wt@x                                [C, N]
  wT = x^T @ (Wt@Wv)^T                    [N, C]   (M = Wt@Wv on host)
  E = exp(q^T k - K_SOFT)                 constant shift instead of row max
  rs[n] = sum_m E[n, m]; recip = 1/rs
  wTs[n, o] = wT[n, o] * recip[n]         (folds the row softmax divide)
  colsum[m] = sum_n recip[n] E[n, m]      (bf16 accumulation on DVE)
  rb[m] = 1 / (1e-9 + colsum[m])
  Wt@x_r = (wTs^T @ E) * rb + bw          (bw = Wt@bv; colsum*rb ~= 1)
  z = (u - bw) - (wTs^T E) * rb           so  x_z = alpha*z + (alpha*bt+beta)
  stats: per-channel sum(z), sum(z^2) transformed to sum(xz), sum(xz^2)
         algebraically, AllReduce'd over the 8 cores PER CHANNEL HALF so
         the first AllReduce + half the BN drain hide under the second
         half's attention-apply compute.
  out = x + relu(Af*z + Bf)  with Af/Bf folding alpha/ab/gamma/mean/var.

Key scheduling facts (from perfetto/NTFF traces of earlier versions):
  - matmul cadence is ~263ns per 512-row instruction for f32r AND bf16
    (1 col/cycle both); only fp8+DoubleRow is faster, and fp8 E flushes
    small attention weights to zero which breaks the double
    normalization (measured 10% error). So the PE work is fixed and the
    wins are structural.
  - the Wt@(x-x_r) stage is folded into the apply matmul via M=Wt@Wv,
    so after the last matmul only a short DVE chain + stats remain.
  - one 2KB AllReduce for the BN stats (a split pair serializes on the
    single CC stream and loses); the alpha/beta/bt affine is folded
    into the stats transform and the BN coefficients, so no separate
    x_z pass exists at all.
  - ALL input DMAs ride one queue in consumption order (a single
    dma_start uses all 16 DMA engines; parallel queues split bandwidth
    and starve the projection pipeline).
  - dummy "preheat" matmuls during the DMA window hold the PE p-state
    ramp (cold cadence is ~427-512ns vs 263ns warm).
  - DVE [P,2048] bf16: tensor_scalar/tensor_tensor are the fast paths
    (~0.5-0.7us); scalar_tensor_tensor and reduce are 1 elem/cycle
    (~2.3us) - the colsum accumulation must use the TS+TT pair.
  - exp keeps accum_out for the row sums: every alternative rowsum
    placement (DVE reduce, gpsimd) is slower or unsupported.

NOTE: walrus --enable-ldw-opt=true crashes codegen on the f32r weight
loads (visitInstLdweights) - it must stay off.
"""

import sys

for _p in ("/opt/trn_rl_repo",):
    if _p not in sys.path:
        sys.path.append(_p)

import numpy as np

import concourse.bass as bass
import concourse.mybir as mybir
import concourse.tile as tile
from concourse import bacc
import concourse.bass_utils as _bu
from concourse.bass_utils import run_bass_kernel_spmd

F32 = mybir.dt.float32
F32R = mybir.dt.float32r
BF16 = mybir.dt.bfloat16
AL = mybir.AluOpType
AF = mybir.ActivationFunctionType
AX = mybir.AxisListType

B, C, N = 8, 256, 2048
P = 128
CB = C // P          # 2 channel blocks
NB = N // P          # 16 row blocks
NQ = N // 512        # 4 column chunks of 512
K_SOFT = 64.0
BN_EPS = 1e-5
DENOM = 1.0 / (B * N)
N_CORES = 8

# vpack vector slot indices
VI = {"bt": 0, "gam": 1, "bnb": 2, "al": 3, "be": 4, "bw": 5}
# wpack projection slot indices
WI = {"q": 0, "k": 1, "m": 2, "t": 3}


def _build_body(tc, x_d, w_d, v_d, out_d, dbg=None):
    nc = tc.nc

    def dump(name, ap):
        if dbg is not None and name in dbg:
            nc.sync.dma_start(dbg[name], ap)

    with (
        tc.tile_pool(name="pp", bufs=1) as pp,
        tc.tile_pool(name="wp", bufs=2) as wp,
        tc.tile_pool(name="dramp", bufs=1, space="DRAM") as dramp,
    ):
        # ---- warmup collective ------------------------------------------
        # a tiny AllReduce issued at kernel start absorbs the
        # first-collective CC-stream setup cost concurrently with compute
        warm_s = pp.tile([1, 8], F32)
        nc.vector.memset(warm_s, 0.0)
        warm_in = dramp.tile([1, 8], F32, name="warm_in")
        warm_out = dramp.tile([1, 8], F32, addr_space="Shared", name="warm_out")
        nc.sync.dma_start(warm_in, warm_s)
        nc.gpsimd.collective_compute(
            "AllReduce",
            AL.add,
            replica_groups=[list(range(N_CORES))],
            ins=[warm_in.opt()],
            outs=[warm_out.opt()],
        )

        # ---- input DMAs (chunk-major, >=4KB contiguous runs) ------------
        x_s = pp.tile([P, NQ, CB * 512], F32R, name="x_s")
        xf_s = x_s.bitcast(F32)
        xp = x_d.rearrange("p (q r) -> p q r", q=NQ)
        wpack = pp.tile([P, 4, CB, C], F32R, name="wpack")
        wsrc = w_d.rearrange("p (w cb o) -> p w cb o", w=4, cb=CB)
        vpack = pp.tile([P, 6, CB], F32, name="vpack")

        # ALL input DMAs ride ONE queue in exact consumption order: a single
        # dma_start already spreads its descriptors across all 16 DMA
        # engines at full HBM bandwidth, while parallel queues SPLIT the
        # bandwidth three ways and make the piece the PE needs next arrive
        # 3x later (measured: q/k stalled ~7us waiting for x chunks).
        nc.sync.dma_start(wpack[:, 0:2], wsrc[:, 0:2])            # Wq, Wk
        nc.sync.dma_start(x_s[:, 0], xp[:, 0])
        nc.sync.dma_start(x_s[:, 1], xp[:, 1])
        nc.sync.dma_start(x_s[:, 2], xp[:, 2])
        nc.sync.dma_start(x_s[:, 3], xp[:, 3])
        nc.sync.dma_start(wpack[:, 2:4], wsrc[:, 2:4])            # M, Wt
        nc.sync.dma_start(vpack, v_d.rearrange("p (v cb) -> p v cb", v=6))

        bt_s = vpack[:, VI["bt"]]
        gam_s = vpack[:, VI["gam"]]
        bnb_s = vpack[:, VI["bnb"]]
        al_s = vpack[:, VI["al"]]
        be_s = vpack[:, VI["be"]]
        bw_s = vpack[:, VI["bw"]]

        # ---- constants --------------------------------------------------
        ones_col_b = pp.tile([P, 1], BF16)
        nc.vector.memset(ones_col_b, 1.0)
        negk_bias = pp.tile([P, 1], F32)
        nc.vector.memset(negk_bias, -K_SOFT)
        eps_bias = pp.tile([P, 1], F32)
        nc.vector.memset(eps_bias, BN_EPS)

        # per-channel constants for the stats transform:
        #   s1 = al*sum_z + N*ab
        #   s2 = al^2*sum_z2 + 2*al*ab*sum_z + N*ab^2
        ab_s = pp.tile([P, CB], F32)        # alpha*bt + beta
        al2_s = pp.tile([P, CB], F32)       # alpha^2
        talab_s = pp.tile([P, CB], F32)     # 2*alpha*ab
        c1_s = pp.tile([P, CB], F32)        # N*ab
        c2_s = pp.tile([P, CB], F32)        # N*ab^2
        nc.vector.tensor_tensor(ab_s, al_s, bt_s, AL.mult)
        nc.vector.tensor_tensor(ab_s, ab_s, be_s, AL.add)
        nc.vector.tensor_tensor(al2_s, al_s, al_s, AL.mult)
        nc.vector.tensor_tensor(talab_s, al_s, ab_s, AL.mult)
        nc.vector.tensor_scalar_mul(talab_s, talab_s, 2.0)
        nc.vector.tensor_scalar_mul(c1_s, ab_s, float(N))
        nc.vector.tensor_tensor(c2_s, ab_s, ab_s, AL.mult)
        nc.vector.tensor_scalar_mul(c2_s, c2_s, float(N))

        # ---- P1: projections q, k, u (Wt@x), wT (x^T M^T) ---------------
        # q/k live only through the energy loop; a released pool frees their
        # 32KB/partition for the drain pipeline buffers afterwards
        qkp = tc.alloc_tile_pool(name="qkp", bufs=1)
        q_s = qkp.tile([P, CB, N], F32R, name="q_s")
        k_s = qkp.tile([P, CB, N], F32R, name="k_s")
        u_s = pp.tile([P, CB, N], F32, name="u_s")
        wT_s = pp.tile([P, NB, C], BF16, name="wT_s")
        # PE preheat: dummy matmuls on zeroed scratch keep the Tensor engine
        # busy through the input-DMA window so the p-state ramp (full clock
        # needs ~3us of continuous execution) completes before the real
        # projections start; without it the first ~7 matmuls after every
        # cold start run at the mid p-state (427ns vs 263ns per 512 rows).
        ph_w = pp.tile([P, P], BF16, name="ph_w")
        ph_r = pp.tile([P, 512], BF16, name="ph_r")
        nc.vector.memset(ph_w, 0.0)
        nc.vector.memset(ph_r, 0.0)
        with tc.tile_pool(name="psA", bufs=4, space="PSUM") as psA:
            ph_p = psA.tile([P, 512], F32, tag="ph", bufs=1, name="ph_p")
            for _ in range(13):
                nc.tensor.matmul(ph_p, ph_w, ph_r, start=True, stop=True)
            for ch in range(NQ):
                sl = slice(ch * 512, (ch + 1) * 512)
                for ob in range(CB):
                    pq = psA.tile([P, 512], F32, tag="pqk", name="pq")
                    for ci in range(CB):
                        nc.tensor.matmul(
                            pq,
                            wpack[:, WI["q"], ci, ob * P : (ob + 1) * P],
                            x_s[:, ch, ci * 512 : (ci + 1) * 512],
                            start=(ci == 0),
                            stop=(ci == CB - 1),
                        )
                    pk = psA.tile([P, 512], F32, tag="pqk", name="pk")
                    for ci in range(CB):
                        nc.tensor.matmul(
                            pk,
                            wpack[:, WI["k"], ci, ob * P : (ob + 1) * P],
                            x_s[:, ch, ci * 512 : (ci + 1) * 512],
                            start=(ci == 0),
                            stop=(ci == CB - 1),
                        )
                    nc.any.tensor_copy(q_s[:, ob, sl], pq)
                    nc.any.tensor_copy(k_s[:, ob, sl], pk)
            for ch in range(NQ):
                for j in range(4):
                    nb = ch * 4 + j
                    pw = psA.tile([P, C], F32, tag="pw", bufs=2, name="pw")
                    for ci in range(CB):
                        nc.tensor.matmul(
                            pw,
                            x_s[:, ch, ci * 512 + j * P : ci * 512 + (j + 1) * P],
                            wpack[:, WI["m"], ci, :],
                            start=(ci == 0),
                            stop=(ci == CB - 1),
                        )
                    nc.any.tensor_copy(wT_s[:, nb, :], pw)

        dump("q_s", q_s)
        dump("k_s", k_s)
        dump("u_s", u_s)
        dump("wT_s", wT_s)

        # ---- P2: energy -> exp -> row normalizers -----------------------
        E_s = pp.tile([P, NB, N], BF16, name="E_s")
        wTs_s = pp.tile([P, NB, C], BF16, name="wTs_s")
        acc_s = pp.tile([P, N], BF16, name="acc_s")
        recip_s = pp.tile([P, NB], F32, name="recip_s")
        with tc.tile_pool(name="psE", bufs=2, space="PSUM") as psE:
            for i in range(NB):
                pe = psE.tile([P, N], F32, tag="e", name="pe")
                for cb in range(CB):
                    for qd in range(NQ):
                        nc.tensor.matmul(
                            pe[:, qd * 512 : (qd + 1) * 512],
                            q_s[:, cb, i * P : (i + 1) * P],
                            k_s[:, cb, qd * 512 : (qd + 1) * 512],
                            start=(cb == 0),
                            stop=(cb == CB - 1),
                        )
                # rowsum via the activation accumulator (a DVE reduce of
                # [P,2048]bf16 is the SLOW 1-elem/cycle path - 2.3us - and
                # GpSimd only reduces along partitions); the TS+TT pair for
                # the colsum accumulation is likewise 2x faster than the
                # fused scalar_tensor_tensor on this input shape.
                rs = wp.tile([P, 1], F32, tag="rs", name="rs")
                nc.scalar.activation(
                    E_s[:, i, :], pe, AF.Exp, bias=negk_bias, accum_out=rs
                )
                nc.vector.reciprocal_approx_fast(recip_s[:, i : i + 1], rs)
                nc.vector.tensor_scalar_mul(
                    wTs_s[:, i, :], wT_s[:, i, :], recip_s[:, i : i + 1]
                )
                if i == 0:
                    nc.vector.tensor_scalar(
                        acc_s, E_s[:, i, :], recip_s[:, i : i + 1], None, AL.mult
                    )
                else:
                    En = wp.tile([P, N], BF16, tag="En", name="En")
                    nc.vector.tensor_scalar(
                        En, E_s[:, i, :], recip_s[:, i : i + 1], None, AL.mult
                    )
                    nc.vector.tensor_tensor(acc_s, acc_s, En, AL.add)

            # u projection INSIDE the psE pool, on tag-"e" tiles: the pool
            # cycles 2 buffers, so these allocations wait only on exp_14's
            # buffer (free ~2us before exp_15's) and the PE rolls straight
            # through the energy->apply boundary instead of idling ~2.7us
            # on the exp_15 PSUM handoff.
            for ch in range(NQ):
                usl = slice(ch * 512, (ch + 1) * 512)
                for ob in range(CB):
                    pu = psE.tile([P, N], F32, tag="e", name="pu")
                    for ci in range(CB):
                        nc.tensor.matmul(
                            pu[:, 0:512],
                            wpack[:, WI["t"], ci, ob * P : (ob + 1) * P],
                            x_s[:, ch, ci * 512 : (ci + 1) * 512],
                            start=(ci == 0),
                            stop=(ci == CB - 1),
                        )
                    nc.any.tensor_copy(u_s[:, ob, usl], pu[:, 0:512])

        dump("E_s", E_s)
        dump("wTs_s", wTs_s)
        dump("recip_s", recip_s)

        qkp.release()

        with tc.tile_pool(name="psX", bufs=4, space="PSUM") as psX:
            rb_s = pp.tile([P, N], F32, name="rb_s")
            z_s = pp.tile([P, CB, N], F32, name="z_s")
            zl1 = pp.tile([P, CB, NQ], F32, name="zl1")
            zl2 = pp.tile([P, CB, NQ], F32, name="zl2")

            def apply_mm(ob, qd):
                sl = slice(qd * 512, (qd + 1) * 512)
                px = psX.tile([P, 512], F32, tag="px", name="px")
                for i in range(NB):
                    nc.tensor.matmul(
                        px,
                        wTs_s[:, i, ob * P : (ob + 1) * P],
                        E_s[:, i, sl],
                        start=(i == 0),
                        stop=(i == NB - 1),
                    )
                return px

            def apply_ep(ob, qd, px):
                sl = slice(qd * 512, (qd + 1) * 512)
                t1 = wp.tile([P, 512], F32, tag="t1", bufs=3, name="t1")
                nc.vector.tensor_tensor(t1, px, rb_s[:, sl], AL.mult)
                # z = (u - bw) - t1 ; accumulate per-channel sum(z)
                nc.vector.scalar_tensor_tensor(
                    z_s[:, ob, sl],
                    u_s[:, ob, sl],
                    bw_s[:, ob : ob + 1],
                    t1,
                    AL.subtract,
                    AL.subtract,
                    accum_out=zl1[:, ob, qd : qd + 1],
                )
                # sum(z^2) on the Scalar engine (idle during apply)
                sq = wp.tile([P, 512], F32, tag="sq", bufs=3, name="sq")
                nc.scalar.activation(
                    sq,
                    z_s[:, ob, sl],
                    AF.Square,
                    accum_out=zl2[:, ob, qd : qd + 1],
                )

            # the first two apply chunks' matmuls are emitted before the
            # column-normalizer work so the PE rolls straight into the apply
            # chain and the colsum matmuls (which wait on the DVE acc chain)
            # never bubble the PE (the epilogues, which read rb_s, are
            # emitted after colsum so Tile orders the writes first)
            px00 = apply_mm(0, 0)
            px01 = apply_mm(0, 1)

            # ---- column normalizer rb = 1/(1e-9 + colsum), broadcast ----
            for qd in range(NQ):
                sl = slice(qd * 512, (qd + 1) * 512)
                pcs = psX.tile([1, 512], F32, tag="cs", bufs=2, name="pcs")
                nc.tensor.matmul(pcs, ones_col_b, acc_s[:, sl], start=True, stop=True)
                rt = wp.tile([1, 512], F32, tag="rt", bufs=1, name="rt")
                nc.vector.tensor_scalar_add(rt, pcs, 1e-9)
                rb1 = wp.tile([1, 512], F32, tag="rb1", bufs=1, name="rb1")
                nc.vector.reciprocal_approx_fast(rb1, rt)
                nc.gpsimd.partition_broadcast(rb_s[:, sl], rb1)

            dump("rb_s", rb_s)

            # ---- apply + per-channel-half stats, pipelined AllReduces ---
            # AR(ob0) triggers ~17us before AR(ob1): it absorbs the core-
            # launch skew + ring latency while the ob1 apply matmuls run, so
            # AR(ob1) starts on a free CC stream between already-synced
            # cores (~14us ring only). The ob0 BN drain then hides inside
            # AR(ob1)'s window; only the ob1 half-drain stays exposed.
            # Partial stat reduces for qd0..2 are emitted before the last
            # chunk so the post-last-matmul chain is just add+3xSTT+DMA.
            stats_t, sin_d, sout_d = [], [], []
            for ob in range(CB):
                stats_t.append(pp.tile([P, 2], F32, name=f"stats{ob}"))
                sin_d.append(dramp.tile([P, 2], F32, name=f"sin{ob}"))
                sout_d.append(
                    dramp.tile([P, 2], F32, addr_space="Shared", name=f"sout{ob}")
                )
            szp = pp.tile([P, 2 * CB], F32, name="szp")  # partial/final sums

            def ob_stats(ob):
                # finalize sums: partial(qd0..2) + last chunk's column
                nc.vector.tensor_tensor(
                    szp[:, 2 * ob : 2 * ob + 1],
                    szp[:, 2 * ob : 2 * ob + 1],
                    zl1[:, ob, 3:4],
                    AL.add,
                )
                nc.vector.tensor_tensor(
                    szp[:, 2 * ob + 1 : 2 * ob + 2],
                    szp[:, 2 * ob + 1 : 2 * ob + 2],
                    zl2[:, ob, 3:4],
                    AL.add,
                )
                st = stats_t[ob]
                szt = szp[:, 2 * ob : 2 * ob + 1]
                sz2t = szp[:, 2 * ob + 1 : 2 * ob + 2]
                # s1 = al*sum_z + N*ab
                nc.vector.scalar_tensor_tensor(
                    st[:, 0:1], szt, al_s[:, ob : ob + 1], c1_s[:, ob : ob + 1],
                    AL.mult, AL.add,
                )
                # s2 = al^2*sum_z2 + (2*al*ab*sum_z + N*ab^2)
                ta = wp.tile([P, 1], F32, tag="ta", name="ta")
                nc.vector.scalar_tensor_tensor(
                    ta, szt, talab_s[:, ob : ob + 1], c2_s[:, ob : ob + 1],
                    AL.mult, AL.add,
                )
                nc.vector.scalar_tensor_tensor(
                    st[:, 1:2], sz2t, al2_s[:, ob : ob + 1], ta, AL.mult, AL.add
                )
                nc.gpsimd.dma_start(sin_d[ob], st)
                nc.gpsimd.collective_compute(
                    "AllReduce",
                    AL.add,
                    replica_groups=[list(range(N_CORES))],
                    ins=[sin_d[ob].opt()],
                    outs=[sout_d[ob].opt()],
                )

            apply_ep(0, 0, px00)
            apply_ep(0, 1, px01)
            for ob in range(CB):
                for qd in range(NQ):
                    if ob == 0 and qd < 2:
                        continue
                    apply_ep(ob, qd, apply_mm(ob, qd))
                    if qd == 2:
                        # partial sums over qd0..2 while qd3's matmuls run
                        nc.vector.reduce_sum(
                            szp[:, 2 * ob : 2 * ob + 1], zl1[:, ob, 0:3], axis=AX.X
                        )
                        nc.vector.reduce_sum(
                            szp[:, 2 * ob + 1 : 2 * ob + 2],
                            zl2[:, ob, 0:3],
                            axis=AX.X,
                        )
                ob_stats(ob)

            dump("z_s", z_s)

            # ---- BN coefficients + drain, per channel half --------------
            # emitted AFTER both apply halves so the drain-ob0 instructions
            # never head-of-line-block the ob1 epilogues; at execution time
            # drain-ob0 runs during AR(ob1)'s window.
            op = out_d.rearrange("p (cb n) -> p cb n", cb=CB)
            Af_s = pp.tile([P, CB], F32, name="Af_s")
            Bf_s = pp.tile([P, CB], F32, name="Bf_s")
            with tc.tile_pool(name="ep", bufs=8) as ep:
                for ob in range(CB):
                    rstat = pp.tile([P, 2], F32, name=f"rstat{ob}")
                    nc.sync.dma_start(rstat, sout_d[ob])
                    mv = wp.tile([P, 2], F32, tag="mv", name="mv")
                    nc.vector.tensor_scalar_mul(mv, rstat, DENOM)
                    mean = mv[:, 0:1]
                    m2 = wp.tile([P, 1], F32, tag="m2", name="m2")
                    nc.vector.tensor_tensor(m2, mean, mean, AL.mult)
                    varr = wp.tile([P, 1], F32, tag="varr", name="varr")
                    nc.vector.tensor_tensor(varr, mv[:, 1:2], m2, AL.subtract)
                    sd = wp.tile([P, 1], F32, tag="sd", name="sd")
                    nc.scalar.activation(sd, varr, AF.Sqrt, bias=eps_bias)
                    inv = wp.tile([P, 1], F32, tag="inv", name="inv")
                    nc.vector.reciprocal(inv, sd)
                    a0 = wp.tile([P, 1], F32, tag="a0", name="a0")
                    nc.vector.tensor_tensor(a0, gam_s[:, ob : ob + 1], inv, AL.mult)
                    nc.vector.tensor_tensor(
                        Af_s[:, ob : ob + 1], a0, al_s[:, ob : ob + 1], AL.mult
                    )
                    tmb = wp.tile([P, 1], F32, tag="tmb", name="tmb")
                    nc.vector.tensor_tensor(
                        tmb, ab_s[:, ob : ob + 1], mean, AL.subtract
                    )
                    nc.vector.tensor_tensor(tmb, a0, tmb, AL.mult)
                    nc.vector.tensor_tensor(
                        Bf_s[:, ob : ob + 1], tmb, bnb_s[:, ob : ob + 1], AL.add
                    )
                    for qd in range(NQ):
                        sl = slice(qd * 512, (qd + 1) * 512)
                        xn = ep.tile([P, 512], F32, tag="xn", name="xn")
                        nc.scalar.activation(
                            xn,
                            z_s[:, ob, sl],
                            AF.Relu,
                            bias=Bf_s[:, ob : ob + 1],
                            scale=Af_s[:, ob : ob + 1],
                        )
                        oc = ep.tile([P, 512], F32, tag="oc", name="oc")
                        nc.vector.tensor_tensor(
                            oc, xn, xf_s[:, qd, ob * 512 : (ob + 1) * 512], AL.add
                        )
                        (nc.sync if qd % 2 == 0 else nc.gpsimd).dma_start(
                            op[:, ob, sl], oc
                        )


def build():
    nc = bacc.Bacc(
        "TRN2", target_bir_lowering=False, debug=False, num_devices=N_CORES
    )
    x_d = nc.dram_tensor("x", [P, NQ * CB * 512], F32R, kind="ExternalInput").ap()
    w_d = nc.dram_tensor("wpack", [P, 4 * CB * C], F32R, kind="ExternalInput").ap()
    v_d = nc.dram_tensor("vpack", [P, 6 * CB], F32, kind="ExternalInput").ap()
    out_d = nc.dram_tensor("out", [P, CB * N], F32, kind="ExternalOutput").ap()
    with tile.TileContext(nc) as tc:
        _build_body(tc, x_d, w_d, v_d, out_d)
    nc.compile()
    return nc


_NC_CACHE = None


def _get_nc():
    global _NC_CACHE
    if _NC_CACHE is None:
        _NC_CACHE = build()
    return _NC_CACHE


def pack_inputs(inputs):
    f = lambda k: np.asarray(inputs[k], dtype=np.float32)
    x = f("x")
    # [C, N] -> [P, NQ, CB, 512] chunk-major -> [P, NQ*CB*512]
    xp = [
        np.ascontiguousarray(
            x[b]
            .reshape(CB, P, NQ, 512)
            .transpose(1, 2, 0, 3)
            .reshape(P, NQ * CB * 512)
        )
        for b in range(B)
    ]
    Wt64 = f("Wt").astype(np.float64)
    M = (Wt64 @ f("Wv").astype(np.float64)).astype(np.float32)
    bw = (Wt64 @ f("bv").astype(np.float64)).astype(np.float32)
    wts = np.stack([f("Wq").T, f("Wk").T, M.T, f("Wt").T])  # [4, C(in), C(out)]
    wpack = np.ascontiguousarray(
        wts.reshape(4, CB, P, C).transpose(2, 0, 1, 3).reshape(P, 4 * CB * C)
    )
    vecs = np.stack(
        [
            f("bt"),
            f("bn_gamma"),
            f("bn_beta"),
            f("alpha").reshape(C),
            f("beta").reshape(C),
            bw,
        ]
    )  # [6, C]
    vpack = np.ascontiguousarray(
        vecs.reshape(6, CB, P).transpose(2, 0, 1).reshape(P, 6 * CB)
    )
    shared = {"wpack": wpack, "vpack": vpack}
    return xp, shared


def kernel(**inputs):
    xp, shared = pack_inputs(inputs)
    nc = _get_nc()
    in_maps = [dict(shared, x=xp[b]) for b in range(B)]
    res = run_bass_kernel_spmd(nc, in_maps, core_ids=list(range(N_CORES)))
    out = np.stack([res.results[b]["out"] for b in range(B)], axis=0)
    # [B, P, CB*N] -> [B, C, N]
    return np.ascontiguousarray(
        out.reshape(B, P, CB, N).transpose(0, 2, 1, 3).reshape(B, C, N)
    )
